# revision 1
# baseline (speedup 1.0000x reference)
"""GGNN message passing + bilinear readout on 8 TRN2 NeuronCores — v2.

Destination-sharded, aggregate-then-transform:
    core k owns dst nodes t_k = [256k, 256k+256) and edge[:, t_k, :].
    Each iteration:
      1. one remote_dma_broadcast per core: own h shard (natural layout,
         [128, 256] = 2 row-chunks side by side) lands at slot k of every
         core's htf buffer (parity-alternating to avoid cross-round races).
      2. Ag_e = edge_e^T @ h   (aggregate senders first; edge channels
         packed in pairs for 512-wide matmuls; slot-major chunk order with
         per-slot arrival gating so aggregation starts while peer shards
         are still in flight).
      3. msgsT = sum_e W_e^T @ Ag_e  (8 accumulating matmuls @256) — the
         [N, E, D] per-edge intermediate of the reference never
         materializes, and no cross-core reduce is needed.
      4. GRU in transposed layout, then a PE transpose of the new own h
         shard back to natural layout for the next round's broadcast.
    Round 9 broadcasts own h_final^T (transposed) straight into the
    readout's gather buffer; logits rows = (h_k A) @ h^T.

Synchronization notes: Tile is a dataflow scheduler — bare sem waits
float. Every wait lives in a tile_critical together with a tiny
self-copy "touch" whose outs overlap the guarded region, giving
downstream readers (or writers, WAR) a true data dep on the critical.
Senders use per-slot receive semaphores (slot j bumps rsem[j][parity])
so each Ag slot can be gated individually.
"""

import sys

for _p in ("/opt/trn_rl_repo",):
    if _p not in sys.path:
        sys.path.insert(0, _p)

import numpy as np
import ml_dtypes

import concourse.bacc as bacc
import concourse.tile as tile
import concourse.mybir as mybir
from concourse import bass_utils

dt = mybir.dt
AF = mybir.ActivationFunctionType

N_CORES = 8
N = 2048
D = 128
E = 8
ITERS = 8
S = N // N_CORES          # 256 own nodes
C = N // D                # 16 sender chunks of 128


def build_nc2(reps=1, skip_exch=False, iters=ITERS, no_waits=False,
              slot_gate=True, pool_mode="stack"):
    nc = bacc.Bacc("TRN2", target_bir_lowering=False, debug=False,
                   num_devices=N_CORES)

    edgek = nc.dram_tensor("edgek", [D, E * C * S], dt.bfloat16,
                           kind="ExternalInput")
    h0t = nc.dram_tensor("h0t", [D, S], dt.bfloat16, kind="ExternalInput")
    h0n = nc.dram_tensor("h0n", [D, S], dt.bfloat16, kind="ExternalInput")
    wmsg = nc.dram_tensor("wmsg", [D, E * D], dt.bfloat16, kind="ExternalInput")
    wi = nc.dram_tensor("wi", [D, 3 * D], dt.bfloat16, kind="ExternalInput")
    wh = nc.dram_tensor("wh", [D, 3 * D], dt.bfloat16, kind="ExternalInput")
    bias = nc.dram_tensor("bias", [D, 3], dt.float32, kind="ExternalInput")
    aro = nc.dram_tensor("aro", [D, D], dt.bfloat16, kind="ExternalInput")
    idn = nc.dram_tensor("idn", [D, D], dt.bfloat16, kind="ExternalInput")
    out = nc.dram_tensor("out", [S, N], dt.float32, kind="ExternalOutput")

    htf = [nc.alloc_sbuf_tensor(f"htf{p}", [D, N], dt.bfloat16)
           for p in range(2)]
    htfT = nc.alloc_sbuf_tensor("htfT", [D, N], dt.bfloat16)
    hsend = nc.alloc_sbuf_tensor("hsend", [D, S], dt.bfloat16)
    htsend = nc.alloc_sbuf_tensor("htsend", [D, S], dt.bfloat16)
    if not skip_exch:
        # Per-slot receive sems x round parity: sender k bumps rsems[k][q%2]
        # on every core; a fast peer's round-(q+1) arrival can't satisfy a
        # round-q wait, and each slot is gated individually.
        rsems = [[nc.alloc_semaphore(name=f"rs{j}_{p}") for p in range(2)]
                 for j in range(N_CORES)]
        lsem = nc.alloc_semaphore(name="lsem")
        psem = nc.alloc_semaphore(name="psem")
    rounds = [0]   # global exchange-round counter (9 per rep)
    preps = [0]

    def prep_round(g, q):
        """Generate the (single) broadcast descriptor for global round q."""
        alldests = [(0, j) for j in range(N_CORES)]
        hq = q % 9                      # 0..7 h-rounds, 8 = hT round
        hpar = (q - q // 9) % 2         # parity over h-rounds only
        pid = g.partition_id()
        for k in g.Switch(pid, N_CORES):
            rs = rsems[k][q % 2]
            if hq < 8:
                g.remote_dma_broadcast(
                    htf[hpar].ap()[:, k * S:(k + 1) * S], hsend.ap(),
                    rs, lsem, rdests=alldests).then_inc(psem, 1)
            else:
                g.remote_dma_broadcast(
                    htfT.ap()[:, k * S:(k + 1) * S], htsend.ap(),
                    rs, lsem, rdests=alldests).then_inc(psem, 1)
        preps[0] += 1

    def gate_slot(g, buf, j, q):
        """Wait for slot j of round q, then touch a 2-element span covering
        the boundary of the slot's two 128-col chunks so readers of either
        chunk get a data dep on this critical."""
        if not no_waits:
            g.wait_ge(rsems[j][q % 2], 2 * (q // 2 + 1))
        ap = buf.ap()[0:1, j * S + D - 1:j * S + D + 1]
        g.tensor_copy(ap, ap)

    with tile.TileContext(nc, pool_alloc_mode=pool_mode) as tc:
        with (
            tc.tile_pool(name="const", bufs=2) as cpool,
            tc.tile_pool(name="sb", bufs=2) as spool,
            tc.tile_pool(name="stage", bufs=4) as stpool,
            tc.tile_pool(name="agg_ps", bufs=1, space="PSUM") as agg_ps,
            tc.tile_pool(name="mm_ps", bufs=2, space="PSUM") as mm_ps,
            tc.tile_pool(name="gru_ps", bufs=2, space="PSUM") as gru_ps,
        ):
            for rep in range(reps):
                edge_sb = {}
                for pair in range(4):
                    w = C * 2 * S
                    t = cpool.tile([D, w], dt.bfloat16, tag=f"edge{pair}")
                    nc.sync.dma_start(t[:], edgek.ap()[:, pair * w:(pair + 1) * w])
                    edge_sb[pair] = t
                wmsg_sb = cpool.tile([D, E * D], dt.bfloat16, tag="wmsg")
                nc.sync.dma_start(wmsg_sb[:], wmsg.ap())
                wi_sb = cpool.tile([D, 3 * D], dt.bfloat16, tag="wi")
                nc.sync.dma_start(wi_sb[:], wi.ap())
                wh_sb = cpool.tile([D, 3 * D], dt.bfloat16, tag="wh")
                nc.sync.dma_start(wh_sb[:], wh.ap())
                bias_sb = cpool.tile([D, 3], dt.float32, tag="bias")
                nc.sync.dma_start(bias_sb[:], bias.ap())
                aro_sb = cpool.tile([D, D], dt.bfloat16, tag="aro")
                nc.sync.dma_start(aro_sb[:], aro.ap())
                idn_sb = cpool.tile([D, D], dt.bfloat16, tag="idn")
                nc.sync.dma_start(idn_sb[:], idn.ap())

                hT = spool.tile([D, S], dt.bfloat16, tag="hT")
                nc.sync.dma_start(hT[:], h0t.ap())
                if not skip_exch and rep > 0:
                    # previous rep's sends out of hsend must have drained
                    with tc.tile_critical(name=f"lw_rep{rep}"):
                        g = nc.gpsimd
                        g.wait_ge(lsem, 16 * 9 * rep)
                        ap = hsend.ap()[0:1, 0:1]
                        g.tensor_copy(ap, ap)
                nc.sync.dma_start(hsend.ap(), h0n.ap())

                for it in range(iters + 1):
                    q = rounds[0]
                    hpar = (q - q // 9) % 2
                    if not skip_exch:
                        with tc.tile_critical(name=f"x{q}",
                                              no_gpsimd_drain=True):
                            g = nc.gpsimd
                            prep_round(g, q)
                            g.wait_ge(psem, preps[0])
                            g.trigger_dma(1)
                            if not slot_gate and not no_waits:
                                for j in range(N_CORES):
                                    g.wait_ge(rsems[j][q % 2],
                                              2 * (q // 2 + 1))
                                g.wait_ge(lsem, 16 * (q + 1))
                    rounds[0] += 1

                    if it == iters:
                        break

                    # --- aggregate senders: Ag_e = edge_e^T @ h ---
                    # slot-major chunk order; each slot gated on arrival.
                    agg_sb = spool.tile([D, E * S], dt.bfloat16, tag="agg")
                    agp = {}
                    for pair in range(4):
                        agt = agg_ps.tile([D, 2 * S], dt.float32,
                                          tag=f"ag{pair}", name=f"ag{pair}")
                        agp[pair] = agt
                    for j in range(N_CORES):
                        if not skip_exch and slot_gate:
                            with tc.tile_critical(name=f"g{q}_{j}",
                                                  no_gpsimd_drain=True):
                                gate_slot(nc.gpsimd, htf[hpar], j, q)
                        for cc in range(2):
                            c = 2 * j + cc
                            for pair in range(4):
                                nc.tensor.matmul(
                                    agp[pair][:],
                                    htf[hpar].ap()[:, c * D:(c + 1) * D],
                                    edge_sb[pair][:, c * 2 * S:(c + 1) * 2 * S],
                                    start=(j == 0 and cc == 0),
                                    stop=(j == N_CORES - 1 and cc == 1),
                                )
                    for pair in range(4):
                        nc.vector.tensor_copy(
                            agg_sb[:, pair * 2 * S:(pair + 1) * 2 * S],
                            agp[pair][:])

                    # --- transform: msgsT = sum_e W_e^T @ Ag_e ---
                    mp = mm_ps.tile([D, S], dt.float32, tag="mm")
                    for e in range(E):
                        nc.tensor.matmul(
                            mp[:], wmsg_sb[:, e * D:(e + 1) * D],
                            agg_sb[:, e * S:(e + 1) * S],
                            start=(e == 0), stop=(e == E - 1),
                        )
                    msgs_bf = spool.tile([D, S], dt.bfloat16, tag="msgsbf")
                    nc.vector.tensor_copy(msgs_bf[:], mp[:])

                    # --- GRU (transposed layout) ---
                    new_hT = spool.tile([D, S], dt.bfloat16, tag="hT")
                    gate = []
                    for gi in range(2):
                        gp = gru_ps.tile([D, S], dt.float32, tag="gru")
                        nc.tensor.matmul(gp[:], wi_sb[:, gi * D:(gi + 1) * D],
                                         msgs_bf[:], start=True, stop=False)
                        nc.tensor.matmul(gp[:], wh_sb[:, gi * D:(gi + 1) * D],
                                         hT[:], start=False, stop=True)
                        gs = stpool.tile([D, S], dt.float32, tag=f"g{gi}")
                        nc.scalar.activation(gs[:], gp[:], AF.Sigmoid,
                                             bias=bias_sb[:, gi:gi + 1])
                        gate.append(gs)
                    r_g, z_g = gate

                    inp = gru_ps.tile([D, S], dt.float32, tag="gru")
                    nc.tensor.matmul(inp[:], wi_sb[:, 2 * D:3 * D], msgs_bf[:],
                                     start=True, stop=True)
                    hnp = gru_ps.tile([D, S], dt.float32, tag="gru")
                    nc.tensor.matmul(hnp[:], wh_sb[:, 2 * D:3 * D], hT[:],
                                     start=True, stop=True)
                    t1 = stpool.tile([D, S], dt.float32, tag="t1")
                    nc.vector.tensor_mul(t1[:], r_g[:], hnp[:])
                    t2 = stpool.tile([D, S], dt.float32, tag="t2")
                    nc.vector.tensor_add(t2[:], t1[:], inp[:])
                    n_sb = stpool.tile([D, S], dt.float32, tag="n")
                    nc.scalar.activation(n_sb[:], t2[:], AF.Tanh,
                                         bias=bias_sb[:, 2:3])
                    d1 = stpool.tile([D, S], dt.float32, tag="d1")
                    nc.vector.tensor_sub(d1[:], hT[:], n_sb[:])
                    d2 = stpool.tile([D, S], dt.float32, tag="d2")
                    nc.vector.tensor_mul(d2[:], z_g[:], d1[:])
                    nc.vector.tensor_add(new_hT[:], n_sb[:], d2[:])
                    hT = new_hT

                    if not skip_exch and slot_gate and not no_waits:
                        # round q's in-flight read of hsend/htsend must end
                        # before the new h lands there
                        with tc.tile_critical(name=f"lw{q}",
                                              no_gpsimd_drain=True):
                            g = nc.gpsimd
                            g.wait_ge(lsem, 16 * (q + 1))
                            buf = htsend if it == iters - 1 else hsend
                            ap = buf.ap()[0:1, 0:1]
                            g.tensor_copy(ap, ap)
                    if it < iters - 1:
                        # natural-layout copy of new h for the next broadcast
                        for cc in range(2):
                            tp = mm_ps.tile([D, D], dt.float32, tag="mm")
                            nc.tensor.matmul(tp[:],
                                             new_hT[:, cc * D:(cc + 1) * D],
                                             idn_sb[:], start=True, stop=True)
                            nc.vector.tensor_copy(
                                hsend.ap()[:, cc * D:(cc + 1) * D], tp[:])
                    else:
                        nc.vector.tensor_copy(htsend.ap(), new_hT[:])

                # --- readout: logits rows = (h_k A) @ h^T ---
                if not skip_exch and slot_gate:
                    q8 = rounds[0] - 1
                    with tc.tile_critical(name=f"gT{q8}",
                                          no_gpsimd_drain=True):
                        g = nc.gpsimd
                        for j in range(N_CORES):
                            gate_slot(g, htfT, j, q8)
                hap = mm_ps.tile([D, S], dt.float32, tag="mm")
                nc.tensor.matmul(hap[:], aro_sb[:], htsend.ap()
                                 if not skip_exch else hT[:],
                                 start=True, stop=True)
                hA_bf = spool.tile([D, S], dt.bfloat16, tag="hA")
                nc.vector.tensor_copy(hA_bf[:], hap[:])

                for isub in range(2):
                    for jc in range(4):
                        lp = mm_ps.tile([D, 2 * S], dt.float32, tag="mm")
                        nc.tensor.matmul(lp[:],
                                         hA_bf[:, isub * D:(isub + 1) * D],
                                         htfT.ap()[:, jc * 2 * S:(jc + 1) * 2 * S],
                                         start=True, stop=True)
                        ost = stpool.tile([D, 2 * S], dt.float32, tag="ost")
                        nc.vector.tensor_copy(ost[:], lp[:])
                        nc.sync.dma_start(
                            out.ap()[isub * D:(isub + 1) * D,
                                     jc * 2 * S:(jc + 1) * 2 * S],
                            ost[:])

    nc.compile()
    return nc


def make_in_maps2(node_embeddings, edge_embeddings, W_msg, b_msg, Wi, Wh,
                  b_gru, A_readout):
    bf16 = ml_dtypes.bfloat16
    wmsg = np.ascontiguousarray(
        W_msg.transpose(1, 0, 2).reshape(D, E * D)).astype(bf16)
    wi_b = np.ascontiguousarray(Wi).astype(bf16)
    wh_b = np.ascontiguousarray(Wh).astype(bf16)
    b_eff = (b_msg.astype(np.float64) @ Wi.astype(np.float64)
             + b_gru.astype(np.float64)).astype(np.float32)
    bias = np.ascontiguousarray(b_eff.reshape(3, D).T)
    aro_b = np.ascontiguousarray(A_readout).astype(bf16)
    idn = np.eye(D, dtype=np.float32).astype(bf16)

    in_maps = []
    for k in range(N_CORES):
        sl = slice(k * S, (k + 1) * S)
        # edgek[p, (pair, c, e%2, t)] = edge[c*128 + p, k*256 + t, e];
        # channel pairs side by side per chunk for 512-wide Ag matmuls
        ek = edge_embeddings[:, sl, :].transpose(2, 0, 1)      # [E, N, S]
        ek = ek.reshape(E, C, D, S).transpose(2, 0, 1, 3)      # [D, E, C, S]
        ek = ek.reshape(D, 4, 2, C, S).transpose(0, 1, 3, 2, 4)
        ek = np.ascontiguousarray(ek.reshape(D, E * C * S)).astype(bf16)
        hk = node_embeddings[sl]                               # [S, D]
        h0t = np.ascontiguousarray(hk.T).astype(bf16)
        h0n = np.ascontiguousarray(
            hk.reshape(2, D, D).transpose(1, 0, 2).reshape(D, S)).astype(bf16)
        in_maps.append({
            "edgek": ek, "h0t": h0t, "h0n": h0n, "wmsg": wmsg, "wi": wi_b,
            "wh": wh_b, "bias": bias, "aro": aro_b, "idn": idn,
        })
    return in_maps


_cache = {}


def kernel(node_embeddings, edge_embeddings, W_msg, b_msg, Wi, Wh, b_gru,
           A_readout):
    if "nc" not in _cache:
        _cache["nc"] = build_nc2(reps=1)
    nc = _cache["nc"]
    in_maps = make_in_maps2(node_embeddings, edge_embeddings, W_msg, b_msg,
                            Wi, Wh, b_gru, A_readout)
    res = bass_utils.run_bass_kernel_spmd(
        nc, in_maps, core_ids=list(range(N_CORES)))
    return np.concatenate([res.results[k]["out"] for k in range(N_CORES)],
                          axis=0)


def build_timed(reps):
    return build_nc2(reps=reps)


# harness/test compatibility aliases
make_in_maps = make_in_maps2
build_nc = build_nc2



# revision 2
# speedup vs baseline: 1.0496x; 1.0496x over previous
"""GGNN message passing + bilinear readout on 8 TRN2 NeuronCores — v3.

Same math/layout as v2 (destination-sharded, aggregate-then-transform),
restructured synchronization:
  - Round 0's all-gather is gone: the full initial h is DMA-loaded into
    htf[0] on every core (it is a kernel input), so only 8 broadcast
    rounds remain (7 h-rounds + 1 hT round for the readout).
  - No per-slot gpsimd gate criticals. Arrival waits ride directly on the
    consuming instructions' wait tables, applied AFTER the TileContext
    exits (the scheduling-pass CoreSim evaluates instruction waits but
    cannot model remote increments, so inline waits would deadlock it).
    An InstMatmult is a single fused LDW+MM at BIR level and bass moves
    matmul waits to the generated LDWEIGHTS, so the wait gates the weight
    (htf) read correctly.
  - Broadcast descriptor prep for send r is hoisted into iteration r's
    compute window (one small gpsimd critical, off the boundary path).
    The boundary cost is a tiny [wait psem; trigger_dma] critical.
  - hsend/htsend WAR guards (in-flight send must finish reading before
    overwrite) ride on the producer instructions of the overwrite (the
    PE transposes / final GRU add), which gate the actual writers through
    engine-queue order and data deps.

Send r (r = 0..6) carries h_{r+1} (output of iteration r): written to
hsend at the end of iteration r, fired at the top of iteration r+1,
landing in htf[(r+1)%2] slots; consumed by iteration r+1. Send 7 carries
h_final^T to htfT for the bilinear readout.
"""

import sys

for _p in ("/opt/trn_rl_repo",):
    if _p not in sys.path:
        sys.path.insert(0, _p)

import numpy as np
import ml_dtypes

import concourse.bacc as bacc
import concourse.tile as tile
import concourse.mybir as mybir
from concourse import bass_utils
from concourse.tile_rust import add_dep_helper

dt = mybir.dt
AF = mybir.ActivationFunctionType

N_CORES = 8
N = 2048
D = 128
E = 8
ITERS = 8
S = N // N_CORES          # 256 own nodes
C = N // D                # 16 sender chunks of 128
SENDS = ITERS             # 7 h-sends + 1 hT send per rep


def build_nc3(reps=1, iters=ITERS):
    nsends = iters           # sends per rep (iters-1 h-sends + 1 hT send)
    nc = bacc.Bacc("TRN2", target_bir_lowering=False, debug=False,
                   num_devices=N_CORES)

    edgek = nc.dram_tensor("edgek", [D, E * C * S], dt.bfloat16,
                           kind="ExternalInput")
    h0t = nc.dram_tensor("h0t", [D, S], dt.bfloat16, kind="ExternalInput")
    h0all = nc.dram_tensor("h0all", [D, N], dt.bfloat16, kind="ExternalInput")
    wmsg = nc.dram_tensor("wmsg", [D, E * D], dt.bfloat16, kind="ExternalInput")
    wi = nc.dram_tensor("wi", [D, 3 * D], dt.bfloat16, kind="ExternalInput")
    wh = nc.dram_tensor("wh", [D, 3 * D], dt.bfloat16, kind="ExternalInput")
    bias = nc.dram_tensor("bias", [D, 3], dt.float32, kind="ExternalInput")
    aro = nc.dram_tensor("aro", [D, D], dt.bfloat16, kind="ExternalInput")
    idn = nc.dram_tensor("idn", [D, D], dt.bfloat16, kind="ExternalInput")
    out = nc.dram_tensor("out", [S, N], dt.float32, kind="ExternalOutput")

    htf = [nc.alloc_sbuf_tensor(f"htf{p}", [D, N], dt.bfloat16)
           for p in range(2)]
    htfT = nc.alloc_sbuf_tensor("htfT", [D, N], dt.bfloat16)
    hsend = nc.alloc_sbuf_tensor("hsend", [D, S], dt.bfloat16)
    htsend = nc.alloc_sbuf_tensor("htsend", [D, S], dt.bfloat16)
    tpad = nc.alloc_sbuf_tensor("tpad", [D, 2], dt.bfloat16)

    rsems = [[nc.alloc_semaphore(name=f"rs{j}_{p}") for p in range(2)]
             for j in range(N_CORES)]
    lsem = nc.alloc_semaphore(name="lsem")
    psem = nc.alloc_semaphore(name="psem")

    # Runtime-only semaphore waits, applied post-scheduling (see docstring).
    deferred_waits = []

    # The tile scheduler cannot see the runtime arrival waits, so it may
    # interleave a later iteration's gated instructions BEFORE the current
    # iteration's send-cone tail on the same engine queue — a guaranteed
    # cross-engine deadlock at runtime. Pin every compute engine's queue to
    # trace order with sync=False dep edges (scheduling-only, no semaphores;
    # a serial engine loses nothing from its natural order).
    _prev = {}

    def ordered(eng, bi):
        if _prev.get(eng) is not None:
            add_dep_helper(bi.ins, _prev[eng].ins, sync=False,
                           reason=f"{eng} queue trace order")
        _prev[eng] = bi
        return bi

    with tile.TileContext(nc) as tc:
        def prep_round(rep, r):
            """Desc-gen for send r of this rep (one gpsimd critical; hoisted
            into iteration r's compute window)."""
            g_idx = nsends * rep + r
            with tc.tile_critical(name=f"prep{g_idx}", no_gpsimd_drain=True):
                g = nc.gpsimd
                alldests = [(0, j) for j in range(N_CORES)]
                pid = g.partition_id()
                for k in g.Switch(pid, N_CORES):
                    rs = rsems[k][g_idx % 2]
                    if r < nsends - 1:
                        bc = g.remote_dma_broadcast(
                            htf[(r + 1) % 2].ap()[:, k * S:(k + 1) * S],
                            hsend.ap(), rs, lsem, rdests=alldests)
                    else:
                        bc = g.remote_dma_broadcast(
                            htfT.ap()[:, k * S:(k + 1) * S],
                            htsend.ap(), rs, lsem, rdests=alldests)
                    bc.then_inc(psem, 1)

        with (
            tc.tile_pool(name="const", bufs=2) as cpool,
            tc.tile_pool(name="sb", bufs=2) as spool,
            tc.tile_pool(name="stage", bufs=4) as stpool,
            tc.tile_pool(name="agg_ps", bufs=1, space="PSUM") as agg_ps,
            tc.tile_pool(name="mm_ps", bufs=2, space="PSUM") as mm_ps,
            tc.tile_pool(name="gru_ps", bufs=2, space="PSUM") as gru_ps,
        ):
            for rep in range(reps):
                edge_sb = {}
                for pair in range(4):
                    w = C * 2 * S
                    t = cpool.tile([D, w], dt.bfloat16, tag=f"edge{pair}")
                    nc.sync.dma_start(t[:], edgek.ap()[:, pair * w:(pair + 1) * w])
                    edge_sb[pair] = t
                wmsg_sb = cpool.tile([D, E * D], dt.bfloat16, tag="wmsg")
                nc.sync.dma_start(wmsg_sb[:], wmsg.ap())
                wi_sb = cpool.tile([D, 3 * D], dt.bfloat16, tag="wi")
                nc.sync.dma_start(wi_sb[:], wi.ap())
                wh_sb = cpool.tile([D, 3 * D], dt.bfloat16, tag="wh")
                nc.sync.dma_start(wh_sb[:], wh.ap())
                bias_sb = cpool.tile([D, 3], dt.float32, tag="bias")
                nc.sync.dma_start(bias_sb[:], bias.ap())
                aro_sb = cpool.tile([D, D], dt.bfloat16, tag="aro")
                nc.sync.dma_start(aro_sb[:], aro.ap())
                idn_sb = cpool.tile([D, D], dt.bfloat16, tag="idn")
                nc.sync.dma_start(idn_sb[:], idn.ap())

                hT = spool.tile([D, S], dt.bfloat16, tag="hT")
                nc.sync.dma_start(hT[:], h0t.ap())
                # full initial h in natural layout replaces round 0's
                # all-gather. It lives in a pool tile (NOT the raw htf
                # tensor): pool-tile DMA completion is tracked by tile, so
                # iteration 0's agg is ordered after the data lands.
                h0a_sb = cpool.tile([D, N], dt.bfloat16, tag="h0a")
                nc.sync.dma_start(h0a_sb[:], h0all.ap())

                hsend_writers = []

                for it in range(iters + 1):
                    if it > 0:
                        # fire send r = it-1 (prepped during iteration it-1)
                        g_idx = nsends * rep + it - 1
                        with tc.tile_critical(name=f"x{g_idx}",
                                              no_gpsimd_drain=True):
                            g = nc.gpsimd
                            # touch-READ the send payload (write goes to the
                            # tpad scratch: a self-copy write would race with
                            # the descriptor's source read). The critical's
                            # tensor-access tracking orders it (and hence the
                            # trigger) after the copies that wrote hsend.
                            tap = (htsend.ap()[0:1, 0:2] if it == iters
                                   else hsend.ap()[0:1, D - 1:D + 1])
                            # touch on DVE: gpsimd bodies are not ordered
                            # across no_gpsimd_drain criticals (8 parallel
                            # Q7s), so successive Pool touches would race
                            nc.vector.tensor_copy(tpad.ap()[0:1, 0:2], tap)
                            g.wait_ge(psem, g_idx + 1)
                            trig = g.trigger_dma(1)
                        for wtr in hsend_writers:
                            add_dep_helper(trig.ins, wtr.ins,
                                           reason="trigger after hsend write")
                        if it < iters:
                            # Arrival-gate critical (v2-style, but one
                            # critical per round on DVE instead of eight on
                            # gpsimd): per-slot wait + boundary touch. The
                            # critical chain places it after the trigger, and
                            # the agg matmuls' AP deps on the touches order
                            # all gated work after it — both for scheduling
                            # (a critical is an engine barrier) and at
                            # runtime (arrival data deps).
                            thr_in = 2 * (g_idx // 2 + 1)
                            with tc.tile_critical(name=f"g{g_idx}",
                                                  no_gpsimd_drain=True):
                                for j in range(N_CORES):
                                    nc.vector.wait_ge(rsems[j][g_idx % 2],
                                                      thr_in)
                                    tap = htf[it % 2].ap()[
                                        0:1, j * S + D - 1:j * S + D + 1]
                                    nc.vector.tensor_copy(tap, tap)

                    if it == iters:
                        break

                    # hoist send-it's descriptor prep into this iteration
                    prep_round(rep, it)

                    # --- aggregate senders: Ag_e = edge_e^T @ h ---
                    # slot-major chunk order; for it>=1 each accumulation
                    # chain gates per slot on the slot's receive semaphore.
                    g_in = nsends * rep + it - 1     # send feeding this iter
                    agg_sb = spool.tile([D, E * S], dt.bfloat16, tag="agg")
                    agp = {}
                    for pair in range(4):
                        agt = agg_ps.tile([D, 2 * S], dt.float32,
                                          tag=f"ag{pair}", name=f"ag{pair}")
                        agp[pair] = agt
                    chain = {pair: [] for pair in range(4)}
                    for j in range(N_CORES):
                        for cc in range(2):
                            c = 2 * j + cc
                            for pair in range(4):
                                hsrc = (h0a_sb[:, c * D:(c + 1) * D]
                                        if it == 0 else
                                        htf[it % 2].ap()[:, c * D:(c + 1) * D])
                                mm = ordered("pe", nc.tensor.matmul(
                                    agp[pair][:],
                                    hsrc,
                                    edge_sb[pair][:, c * 2 * S:(c + 1) * 2 * S],
                                    start=(j == 0 and cc == 0),
                                    stop=(j == N_CORES - 1 and cc == 1),
                                ))
                                chain[pair].append(mm)
                    for pair in range(4):
                        ordered("dve", nc.vector.tensor_copy(
                            agg_sb[:, pair * 2 * S:(pair + 1) * 2 * S],
                            agp[pair][:]))

                    # --- transform: msgsT = sum_e W_e^T @ Ag_e ---
                    mp = mm_ps.tile([D, S], dt.float32, tag="mm")
                    for e in range(E):
                        ordered("pe", nc.tensor.matmul(
                            mp[:], wmsg_sb[:, e * D:(e + 1) * D],
                            agg_sb[:, e * S:(e + 1) * S],
                            start=(e == 0), stop=(e == E - 1),
                        ))
                    msgs_bf = spool.tile([D, S], dt.bfloat16, tag="msgsbf")
                    ordered("dve", nc.vector.tensor_copy(msgs_bf[:], mp[:]))

                    # --- GRU (transposed layout) ---
                    new_hT = spool.tile([D, S], dt.bfloat16, tag="hT")
                    gate = []
                    for gi in range(2):
                        gp = gru_ps.tile([D, S], dt.float32, tag="gru")
                        ordered("pe", nc.tensor.matmul(
                            gp[:], wi_sb[:, gi * D:(gi + 1) * D],
                            msgs_bf[:], start=True, stop=False))
                        ordered("pe", nc.tensor.matmul(
                            gp[:], wh_sb[:, gi * D:(gi + 1) * D],
                            hT[:], start=False, stop=True))
                        gs = stpool.tile([D, S], dt.float32, tag=f"g{gi}")
                        ordered("act", nc.scalar.activation(
                            gs[:], gp[:], AF.Sigmoid,
                            bias=bias_sb[:, gi:gi + 1]))
                        gate.append(gs)
                    r_g, z_g = gate

                    inp = gru_ps.tile([D, S], dt.float32, tag="gru")
                    ordered("pe", nc.tensor.matmul(
                        inp[:], wi_sb[:, 2 * D:3 * D], msgs_bf[:],
                        start=True, stop=True))
                    hnp = gru_ps.tile([D, S], dt.float32, tag="gru")
                    ordered("pe", nc.tensor.matmul(
                        hnp[:], wh_sb[:, 2 * D:3 * D], hT[:],
                        start=True, stop=True))
                    t1 = stpool.tile([D, S], dt.float32, tag="t1")
                    ordered("dve", nc.vector.tensor_mul(t1[:], r_g[:], hnp[:]))
                    t2 = stpool.tile([D, S], dt.float32, tag="t2")
                    ordered("dve", nc.vector.tensor_add(t2[:], t1[:], inp[:]))
                    n_sb = stpool.tile([D, S], dt.float32, tag="n")
                    fin = ordered("act", nc.scalar.activation(
                        n_sb[:], t2[:], AF.Tanh, bias=bias_sb[:, 2:3]))
                    d1 = stpool.tile([D, S], dt.float32, tag="d1")
                    sub_d1 = ordered("dve", nc.vector.tensor_sub(
                        d1[:], hT[:], n_sb[:]))
                    d2 = stpool.tile([D, S], dt.float32, tag="d2")
                    mul_d2 = ordered("dve", nc.vector.tensor_mul(
                        d2[:], z_g[:], d1[:]))
                    fin_add = ordered("dve", nc.vector.tensor_add(
                        new_hT[:], n_sb[:], d2[:]))
                    hT = new_hT

                    # WAR guard: send it-1 must finish reading hsend/htsend
                    # before this iteration's copies overwrite it. The wait
                    # rides on the copy's producer (PE transpose / GRU add),
                    # which gates the copy via its data dependency.
                    lthr = 16 * (nsends * rep + it)
                    if it < iters - 1:
                        writers = []
                        for cc in range(2):
                            tp = mm_ps.tile([D, D], dt.float32, tag="mm")
                            tmm = ordered("pe", nc.tensor.matmul(
                                tp[:], new_hT[:, cc * D:(cc + 1) * D],
                                idn_sb[:], start=True, stop=True))
                            deferred_waits.append((tmm, lsem, lthr))
                            cp = ordered("dve", nc.vector.tensor_copy(
                                hsend.ap()[:, cc * D:(cc + 1) * D], tp[:]))
                            writers.append(cp)
                        hsend_writers = writers
                    else:
                        # htsend WAR vs the previous rep's hT send is implied
                        # transitively: iteration 0's transposes already wait
                        # lsem >= 16*nsends*rep, and this copy is downstream
                        # of them through the data/queue order.
                        cp = ordered("dve", nc.vector.tensor_copy(
                            htsend.ap(), new_hT[:]))
                        hsend_writers = [cp]

                # --- readout: logits rows = (h_k A) @ h^T ---
                g8 = nsends * rep + nsends - 1
                thr8 = 2 * (g8 // 2 + 1)
                hap = mm_ps.tile([D, S], dt.float32, tag="mm")
                hap_mm = ordered("pe", nc.tensor.matmul(
                    hap[:], aro_sb[:], htsend.ap(), start=True, stop=True))
                hA_bf = spool.tile([D, S], dt.bfloat16, tag="hA")
                hA_cp = ordered("dve", nc.vector.tensor_copy(
                    hA_bf[:], hap[:]))

                # readout slot gating: one v2-style gpsimd critical (the
                # per-iteration hot path has none of these; once per rep the
                # cost is negligible). The touches give the lp matmuls a
                # data dep on the gated arrivals.
                with tc.tile_critical(name=f"gT{g8}", no_gpsimd_drain=True):
                    for j in range(N_CORES):
                        nc.vector.wait_ge(rsems[j][g8 % 2], thr8)
                        ap = htfT.ap()[0:1, j * S + D - 1:j * S + D + 1]
                        nc.vector.tensor_copy(ap, ap)
                for isub in range(2):
                    for jc in range(4):
                        lp = mm_ps.tile([D, 2 * S], dt.float32, tag="mm")
                        for jj in range(2):
                            j = 2 * jc + jj
                            ordered("pe", nc.tensor.matmul(
                                lp[:, jj * S:(jj + 1) * S],
                                hA_bf[:, isub * D:(isub + 1) * D],
                                htfT.ap()[:, j * S:(j + 1) * S],
                                start=True, stop=True))
                        ost = stpool.tile([D, 2 * S], dt.float32, tag="ost")
                        ordered("dve", nc.vector.tensor_copy(ost[:], lp[:]))
                        nc.sync.dma_start(
                            out.ap()[isub * D:(isub + 1) * D,
                                     jc * 2 * S:(jc + 1) * 2 * S],
                            ost[:])

    import bass_rust.bass_rust as _br
    # Attach the runtime arrival waits. For a matmul the wait must gate the
    # paired InstLdweights (the weight/htf read happens there, before the
    # matmul dispatches). Stacking beyond one wait per instruction is legal
    # here: Bacc.generate_event_semaphores legalizes multi-wait instructions
    # by splitting them into preceding InstEventSemaphores.
    pos = {}
    for blk in nc.m.functions[0].blocks:
        for i, ins in enumerate(blk.instructions):
            pos[id(ins)] = (blk, i)
    for entry in deferred_waits:
        inst, sem, val = entry[0], entry[1], entry[2]
        if val <= 0:
            continue
        tgt = inst.ins
        if isinstance(tgt, mybir.InstMatmult):
            blk, i = pos[id(tgt)]
            j = i - 1
            while j >= 0 and not isinstance(
                    blk.instructions[j], mybir.InstLdweights):
                j -= 1
            assert j >= 0, f"no LdWeights before {tgt.name}"
            tgt = blk.instructions[j]
        _br.wait_op(tgt, sem, val, "sem-ge", False)
    nc.compile()
    return nc


def make_in_maps3(node_embeddings, edge_embeddings, W_msg, b_msg, Wi, Wh,
                  b_gru, A_readout):
    bf16 = ml_dtypes.bfloat16
    wmsg = np.ascontiguousarray(
        W_msg.transpose(1, 0, 2).reshape(D, E * D)).astype(bf16)
    wi_b = np.ascontiguousarray(Wi).astype(bf16)
    wh_b = np.ascontiguousarray(Wh).astype(bf16)
    b_eff = (b_msg.astype(np.float64) @ Wi.astype(np.float64)
             + b_gru.astype(np.float64)).astype(np.float32)
    bias = np.ascontiguousarray(b_eff.reshape(3, D).T)
    aro_b = np.ascontiguousarray(A_readout).astype(bf16)
    idn = np.eye(D, dtype=np.float32).astype(bf16)
    # h0all[p, c*D + d] = h[c*128 + p, d] — chunk-major natural layout,
    # matching the htf slot layout for all 2048 nodes.
    h0a = np.ascontiguousarray(
        node_embeddings.reshape(C, D, D).transpose(1, 0, 2)
        .reshape(D, N)).astype(bf16)

    in_maps = []
    for k in range(N_CORES):
        sl = slice(k * S, (k + 1) * S)
        # edgek[p, (pair, c, e%2, t)] = edge[c*128 + p, k*256 + t, e]
        ek = edge_embeddings[:, sl, :].transpose(2, 0, 1)      # [E, N, S]
        ek = ek.reshape(E, C, D, S).transpose(2, 0, 1, 3)      # [D, E, C, S]
        ek = ek.reshape(D, 4, 2, C, S).transpose(0, 1, 3, 2, 4)
        ek = np.ascontiguousarray(ek.reshape(D, E * C * S)).astype(bf16)
        hk = node_embeddings[sl]                               # [S, D]
        h0t = np.ascontiguousarray(hk.T).astype(bf16)
        in_maps.append({
            "edgek": ek, "h0t": h0t, "h0all": h0a, "wmsg": wmsg, "wi": wi_b,
            "wh": wh_b, "bias": bias, "aro": aro_b, "idn": idn,
        })
    return in_maps


_cache = {}


def kernel(node_embeddings, edge_embeddings, W_msg, b_msg, Wi, Wh, b_gru,
           A_readout):
    if "nc" not in _cache:
        _cache["nc"] = build_nc3(reps=1)
    nc = _cache["nc"]
    in_maps = make_in_maps3(node_embeddings, edge_embeddings, W_msg, b_msg,
                            Wi, Wh, b_gru, A_readout)
    res = bass_utils.run_bass_kernel_spmd(
        nc, in_maps, core_ids=list(range(N_CORES)))
    return np.concatenate([res.results[k]["out"] for k in range(N_CORES)],
                          axis=0)


def build_timed(reps):
    return build_nc3(reps=reps)


make_in_maps = make_in_maps3
build_nc = build_nc3
